# revision 15
# baseline (speedup 1.0000x reference)
"""Trainium2 Bass kernel for nn_Encoder_Spatio (gnn_message_passing).

Math (validated against the reference):
    h1 = ELU(x @ fc1_w.T + b1)
    h2 = ELU(h1 @ fc2_w.T + b2)
    probs[b,i,j] = sq[b,i] + sk[b,j] + c
where BN (eval) and the Q/K projections + mlp2 halves fold on the host into
    vq, vk in R^256,  c scalar:    sq = h2 @ vq,  sk = h2 @ vk  (+consts).

Sharding: pure data-parallel over B (8 batches -> 8 cores). Each core runs the
identical program on its own batch; no collectives.

Performance structure (error budget is rel 2e-2; measured ~4e-4):
  * fp32r matmuls: 1 PE cycle/row instead of fp32's 4. Inputs/weights land
    as f32 bytes through fp32r-typed DMAs (the DMA is the "rounded"
    producer the BIR verifier wants; the PE rounds internally).
  * fp16 output: the [2048,2048] score matrix goes to DRAM as fp16
    (8.4 MB instead of 16.8 MB -- the kernel is output-DMA-bound) and is
    upcast to fp32 on the host.
  * Layer biases ride into PSUM as rank-1 matmuls (stationary [1,128] bias
    row x moving ones row), so both oc-chunks of a layer live in one
    two-bank PSUM pair and every ELU op processes [128,1024] at once:
    exp and relu on Scalar (Pool/GpSimd tensor ops are software-emulated
    and ~15x slower AND stall concurrent DVE work, so GpSimd does nothing),
    fused (e-1) min r on Vector, which also rounds its fp32r output.
    ELU(z) = min(exp(z)-1, relu(z)) exactly.
  * t0 (the sk+c broadcast) is stored fp16 so the output emission
    tensor_scalar on DVE runs in its 2-byte 4x perf mode; sq stays a
    per-partition f32 scalar AP (exempt from the 2-byte rule).
  * Output ships triangularly: [512x512] quads as soon as a (row-block,
    column-slice) pair exists, then the last row-block group as four full
    [128x2048] rows. Weights land in ONE packed DMA issued FIRST on the
    sync ring (its hardware queue outranks the scalar ring's), then input
    slices 0-1; slices 2-3 ride the scalar ring.
"""

import sys

if "/opt/trn_rl_repo" not in sys.path:
    sys.path.insert(0, "/opt/trn_rl_repo")

import types

import numpy as np


def _ensure_axon_hooks():
    """concourse.bass_utils imports antenv.axon_hooks when tracing is
    requested; this image's antenv package lacks that submodule, which turns
    a skipped-trace fallback into a hard ImportError. Fill the hole with a
    None-hook stub (tracing degrades gracefully) if it's missing."""
    try:
        import antenv.axon_hooks  # noqa: F401
        return
    except ImportError:
        pass
    try:
        import antenv
    except ImportError:
        return
    mod = types.ModuleType("antenv.axon_hooks")
    mod._hook = None

    def set_axon_ntff_profile_hook(hook):
        mod._hook = hook

    def get_axon_ntff_profile_hook():
        return mod._hook

    mod.set_axon_ntff_profile_hook = set_axon_ntff_profile_hook
    mod.get_axon_ntff_profile_hook = get_axon_ntff_profile_hook
    sys.modules["antenv.axon_hooks"] = mod
    antenv.axon_hooks = mod


_ensure_axon_hooks()

from concourse import bass, tile, mybir
from concourse.bass_utils import run_bass_kernel_spmd

B, N, F = 8, 2048, 256      # batch, tokens, feature width (NIN == NHID == 256)
KC = F // 128               # feature chunks of 128 partitions
TS = 4                      # token slices
SW = N // TS                # slice width (512 = max fp32 matmul moving dim)
RBW = 128                   # output row-block width (partition dim)
NRB = N // RBW              # 16 row blocks
RB_PER_TS = SW // RBW       # row blocks per token slice

MM_DT = mybir.dt.float32r   # one-pass fp32 through the PE (4x fp32 rate)
OUT_DT = mybir.dt.float16   # device-side output precision (host upcasts)

F32 = mybir.dt.float32
AF = mybir.ActivationFunctionType
ALU = mybir.AluOpType

# packed-weights column layout (partition-major [128, WPK] f32).
# Biases appear twice: as [1,128] rows on partition 0 (rank-1 bias matmuls)
# next to a 512-wide ones row used as their moving operand.
WC_W1 = 0                   # KC chunks of F columns
WC_W2 = WC_W1 + KC * F
WC_VKB = WC_W2 + KC * F     # KC chunks of 128 columns
WC_VQ = WC_VKB + KC * 128   # 2*KC columns (vq chunk | zeros pairs)
WC_CB = WC_VQ + 2 * KC      # 1 column
WC_BR = WC_CB + 1           # bias rows: b1|b2 on partition 0 (2*F columns)
WC_ONE = WC_BR + 2 * F      # ones row on partition 0 (SW columns)
WPK = WC_ONE + SW


def _split_multiwaits(nc):
    """This walrus build lowers at most one sync-wait per instruction on some
    instruction classes (the TileContext exit drain trips it). Hoist extra
    waits onto preceding single-wait drains on the same engine."""
    for f in nc.m.functions:
        for bb in f.blocks:
            insts = list(bb.instructions)
            out = []
            changed = False
            for inst in insts:
                si = inst.sync_info
                if si is not None and si.on_wait and len(si.on_wait) > 1:
                    waits = list(si.on_wait)
                    for k, w in enumerate(waits[:-1]):
                        d = mybir.InstDrain(name=f"{inst.name}-ws{k}")
                        d.engine = inst.engine
                        d.sync_info = mybir.SyncInfo(on_wait=[w], on_update=[])
                        out.append(d)
                    inst.sync_info = mybir.SyncInfo(
                        on_wait=[waits[-1]], on_update=list(si.on_update)
                    )
                    changed = True
                out.append(inst)
            if changed:
                bb.instructions = out


def _build_program():
    nc = bass.Bass(trn_type="TRN2")

    xt_d = nc.dram_tensor("xt", [F, N], F32, kind="ExternalInput")
    wpk_d = nc.dram_tensor("wpk", [128, WPK], F32, kind="ExternalInput")
    out_d = nc.dram_tensor("out", [N, N], OUT_DT, kind="ExternalOutput")

    with tile.TileContext(nc) as tc:
        with (
            tc.tile_pool(name="wts", bufs=1) as wpool,
            tc.tile_pool(name="xin", bufs=1) as xpool,
            tc.tile_pool(name="eh", bufs=2) as epool,
            tc.tile_pool(name="rh", bufs=2) as rpool,
            tc.tile_pool(name="h1", bufs=2) as h1pool,
            tc.tile_pool(name="h2", bufs=2) as h2pool,
            tc.tile_pool(name="t0", bufs=1) as t0pool,
            tc.tile_pool(name="sqs", bufs=TS) as sqpool,
            tc.tile_pool(name="ob", bufs=8) as opool,
            tc.tile_pool(name="ps2", bufs=3, space="PSUM") as ps2pool,
            tc.tile_pool(name="pst0", bufs=1, space="PSUM") as t0ps_pool,
            tc.tile_pool(name="pssq", bufs=1, space="PSUM") as sqps_pool,
        ):
            # ---- packed weights FIRST on the sync ring (its queue has
            # priority and every matmul waits on this one transfer)
            wstg = wpool.tile([128, WPK], MM_DT, tag="wstg", name="wstg")
            nc.sync.dma_start(wstg[:], wpk_d[:].bitcast(MM_DT))
            w1 = [wstg[:, WC_W1 + k * F:WC_W1 + (k + 1) * F] for k in range(KC)]
            w2 = [wstg[:, WC_W2 + k * F:WC_W2 + (k + 1) * F] for k in range(KC)]
            vkb = [wstg[:, WC_VKB + k * 128:WC_VKB + (k + 1) * 128]
                   for k in range(KC)]
            vq = wstg[:, WC_VQ:WC_VQ + 2 * KC]
            cb = wstg[:, WC_CB:WC_CB + 1].bitcast(F32)
            # bias rows ([1, 128] stationary slices) + their ones moving row
            br = {1: wstg[0:1, WC_BR:WC_BR + F],
                  2: wstg[0:1, WC_BR + F:WC_BR + 2 * F]}
            ones_row = wstg[0:1, WC_ONE:WC_ONE + SW]

            # ---- input slices: 0-1 behind the weights on the sync ring,
            # 2-3 on the scalar ring; one DMA per slice, fp32r-typed
            xstg = {}
            for s, ring in ((0, nc.sync), (1, nc.sync),
                            (2, nc.scalar), (3, nc.scalar)):
                stg = xpool.tile([128, KC, SW], MM_DT, name=f"xs_{s}", tag=f"xs_{s}")
                ring.dma_start(
                    stg[:],
                    xt_d[:, s * SW:(s + 1) * SW].rearrange(
                        "(k p) c -> p k c", k=KC
                    ).bitcast(MM_DT),
                )
                xstg[s] = stg
            xs = {
                s: [xstg[s][:, k, :] for k in range(KC)] for s in range(TS)
            }

            # HAM warmup: dummy matmuls on memset tiles fill the PE during
            # the load window so the first real matmul runs fast
            dmw = wpool.tile([128, 128], F32, tag="dmw", name="dmw")
            dmx = wpool.tile([128, 256], F32, tag="dmx", name="dmx")
            nc.gpsimd.memset(dmw[:], 0.0)
            nc.gpsimd.memset(dmx[:], 0.0)
            dps = t0ps_pool.tile([128, SW], F32, name="dps", tag="pst0")
            for w_i in range(3):
                nc.tensor.matmul(dps[:, 0:256], dmw[:], dmx[:],
                                 start=(w_i == 0), stop=(w_i == 2))

            t0_full = t0pool.tile([128, N], OUT_DT, name="t0_full", tag="t0_full")
            t0 = []        # per-slice views into t0_full [128, SW]
            sq = []        # per-rowblock sq views [128, 1] (f32)
            h1s = {}       # slice -> [128, KC, SW] fp32r tile
            h2s = {}       # slice -> [128, KC, SW] fp32r tile

            def mlp_layer(w, lb, rhs, out_pool, tag):
                """One Linear+ELU layer for a token slice. Both oc-chunks
                accumulate into one two-bank PSUM pair -- the bias arrives
                as a rank-1 matmul (bias row x ones row) -- so each ELU op
                covers [128, 2*SW]:
                  exp, relu on Scalar;  (e-1) min r on Vector -> fp32r.
                rhs: list of KC [128, SW] fp32r APs. Returns the [128,KC,SW]
                ELU tile (chunk k = [:, k, :])."""
                pp = ps2pool.tile([128, KC, SW], F32, name="pp", tag="pp")
                for oc in range(KC):
                    for k in range(KC):
                        nc.tensor.matmul(
                            pp[:, oc, :],
                            w[k][:, oc * 128:(oc + 1) * 128],
                            rhs[k],
                            start=(k == 0),
                            stop=False,
                        )
                    nc.tensor.matmul(
                        pp[:, oc, :],
                        br[lb][:, oc * 128:(oc + 1) * 128],
                        ones_row,
                        start=False,
                        stop=True,
                    )
                e = epool.tile([128, KC, SW], F32, name="e_t")
                r = rpool.tile([128, KC, SW], F32, name="r_t")
                h = out_pool.tile([128, KC, SW], MM_DT, tag=tag, name=tag)
                nc.scalar.activation(e[:], pp[:], AF.Exp)
                nc.scalar.activation(r[:], pp[:], AF.Relu)
                nc.vector.scalar_tensor_tensor(
                    h[:], e[:], -1.0, r[:], ALU.add, ALU.min
                )
                return h

            n_out = 0

            def emit_quad(q, s):
                """One output DMA covering row blocks 4q..4q+3 at column
                slice s: a [128, 4, 512] fp16 SBUF tile lands on the
                [512, 512] DRAM region in a single transfer. The adds run
                on DVE in its 2-byte 4x perf mode (t0 is fp16, sq a
                per-partition f32 scalar, which is exempt), with a few on
                Scalar to balance the engines."""
                nonlocal n_out
                ot = opool.tile([128, RB_PER_TS, SW], OUT_DT, name="out_t", tag="out_t")
                for rb in range(RB_PER_TS):
                    i = q * RB_PER_TS + rb
                    if n_out % 4 == 3:
                        nc.scalar.activation(
                            ot[:, rb, :], t0[s], AF.Identity, bias=sq[i]
                        )
                    else:
                        nc.vector.tensor_scalar(
                            ot[:, rb, :], t0[s], sq[i], None, ALU.add
                        )
                    n_out += 1
                dram = out_d[
                    q * RB_PER_TS * RBW:(q + 1) * RB_PER_TS * RBW,
                    s * SW:(s + 1) * SW,
                ].rearrange("(b p) c -> p b c", b=RB_PER_TS)
                # sync engine is nearly idle; keep output-DMA issue cost
                # off the busy compute engines
                nc.sync.dma_start(dram, ot[:])

            def stage_T0(s):
                """sk[j] + c for column slice s, broadcast to all partitions
                via the replicated-vk matmul, narrowed to fp16 by the Scalar
                copy that adds the constant."""
                pst = t0ps_pool.tile([128, SW], F32, name="ps_t0", tag="pst0")
                for k in range(KC):
                    nc.tensor.matmul(
                        pst[:], vkb[k][:], h2s[s][:, k, :],
                        start=(k == 0), stop=(k == KC - 1),
                    )
                t0s = t0_full[:, s * SW:(s + 1) * SW]
                nc.scalar.activation(t0s, pst[:], AF.Identity, bias=cb)
                t0.append(t0s)

            def stage_SQ(s):
                """sq for this slice's row blocks (fp32r ISA needs even
                moving/output dims, hence the zero-padded vq pairs). All
                row blocks of the slice land in one PSUM tile so a single
                DVE copy moves them to SBUF."""
                qps = sqps_pool.tile([128, 2 * RB_PER_TS], F32, name="qps")
                for rb in range(RB_PER_TS):
                    for k in range(KC):
                        nc.tensor.matmul(
                            qps[:, 2 * rb:2 * rb + 2],
                            h2s[s][:, k, rb * RBW:(rb + 1) * RBW],
                            vq[:, 2 * k:2 * k + 2],
                            start=(k == 0),
                            stop=(k == KC - 1),
                        )
                sqt = sqpool.tile(
                    [128, 2 * RB_PER_TS], F32, tag=f"sq_{s}", name=f"sq_{s}"
                )
                nc.vector.tensor_copy(sqt[:], qps[:])
                for rb in range(RB_PER_TS):
                    sq.append(sqt[:, 2 * rb:2 * rb + 1])

            def stage_A(s):
                h1s[s] = mlp_layer(w1, 1, xs.pop(s), h1pool, "h1")

            def stage_B(s):
                h2s[s] = mlp_layer(w2, 2, [h1s[s][:, k, :] for k in range(KC)],
                                   h2pool, "h2")
                h1s.pop(s)

            def stage_C(s):
                """Scores for slice s, then the triangular emission of every
                newly-possible (row-block quad, column slice) pair. The last
                slice instead ships its row blocks as full [128, 2048] rows
                (4 KB descriptors)."""
                stage_T0(s)
                stage_SQ(s)
                if s == TS - 1:
                    for q in range(TS - 1):
                        emit_quad(q, s)
                    for rb in range(RB_PER_TS):
                        i = s * RB_PER_TS + rb
                        orow = opool.tile(
                            [128, N], OUT_DT, name="out_row", tag="out_t"
                        )
                        if rb % 2 == 0:
                            nc.vector.tensor_scalar(
                                orow[:], t0_full[:], sq[i], None, ALU.add
                            )
                        else:
                            nc.scalar.activation(
                                orow[:], t0_full[:], AF.Identity, bias=sq[i]
                            )
                        nc.sync.dma_start(
                            out_d[i * RBW:(i + 1) * RBW, :], orow[:]
                        )
                else:
                    for q in range(s):
                        emit_quad(q, s)
                    for sp in range(s + 1):
                        emit_quad(s, sp)

            # PE work order: two A-stages lead before the first B so the PE
            # always has independent work queued while each stage's ELU
            # chain (Scalar exp/relu -> Vector min) drains; C stages follow
            # their B immediately so the output stream starts early.
            stage_A(0)
            stage_A(1)
            stage_A(2)
            stage_B(0)
            stage_A(3)
            stage_C(0)
            stage_B(1)
            stage_C(1)
            stage_B(2)
            stage_C(2)
            stage_B(3)
            stage_C(3)

    _split_multiwaits(nc)
    return nc


_prog_cache = {}


def _get_program():
    if "nc" not in _prog_cache:
        _prog_cache["nc"] = _build_program()
    return _prog_cache["nc"]


def kernel(**inputs):
    inp = np.asarray(inputs["inputs"], np.float32)        # [B, N, F]
    fc1_w = np.asarray(inputs["fc1_w"], np.float64)
    fc1_b = np.asarray(inputs["fc1_b"], np.float64)
    fc2_w = np.asarray(inputs["fc2_w"], np.float64)
    fc2_b = np.asarray(inputs["fc2_b"], np.float64)
    bn_g = np.asarray(inputs["bn_g"], np.float64)
    bn_b = np.asarray(inputs["bn_b"], np.float64)
    bn_mean = np.asarray(inputs["bn_mean"], np.float64)
    bn_var = np.asarray(inputs["bn_var"], np.float64)
    wq_w = np.asarray(inputs["wq_w"], np.float64)
    wq_b = np.asarray(inputs["wq_b"], np.float64)
    wk_w = np.asarray(inputs["wk_w"], np.float64)
    wk_b = np.asarray(inputs["wk_b"], np.float64)
    mlp2_w = np.asarray(inputs["mlp2_w"], np.float64)
    mlp2_b = np.asarray(inputs["mlp2_b"], np.float64)

    # Fold BN (eval) into the Q/K projections, then both projections and the
    # mlp2 halves into two R^F vectors + one scalar (exact linear algebra).
    D = wq_w.shape[0]
    s = bn_g / np.sqrt(bn_var + 1e-5)
    t = bn_b - bn_mean * s
    wqf = wq_w * s[None, :]
    bqf = wq_b + wq_w @ t
    wkf = wk_w * s[None, :]
    bkf = wk_b + wk_w @ t
    wk_half, wq_half = mlp2_w[0, :D], mlp2_w[0, D:]
    vq = wqf.T @ wq_half
    vk = wkf.T @ wk_half
    c_total = float(bqf @ wq_half + bkf @ wk_half + mlp2_b[0])

    # packed weights tile, mirroring the WC_* column layout
    wpk = np.zeros((128, WPK), np.float32)
    wpk[:, WC_W1:WC_W1 + KC * F] = np.concatenate(
        [fc1_w.T[k * 128:(k + 1) * 128, :] for k in range(KC)], axis=1
    )
    wpk[:, WC_W2:WC_W2 + KC * F] = np.concatenate(
        [fc2_w.T[k * 128:(k + 1) * 128, :] for k in range(KC)], axis=1
    )
    wpk[:, WC_VKB:WC_VKB + KC * 128] = np.concatenate(
        [np.tile(vk[k * 128:(k + 1) * 128, None], (1, 128)) for k in range(KC)],
        axis=1,
    )
    for k in range(KC):
        wpk[:, WC_VQ + 2 * k] = vq[k * 128:(k + 1) * 128]
    wpk[:, WC_CB] = c_total
    wpk[0, WC_BR:WC_BR + F] = fc1_b
    wpk[0, WC_BR + F:WC_BR + 2 * F] = fc2_b
    wpk[0, WC_ONE:WC_ONE + SW] = 1.0

    in_maps = [
        {"xt": np.ascontiguousarray(inp[b].T), "wpk": wpk} for b in range(B)
    ]

    nc = _get_program()
    res = run_bass_kernel_spmd(nc, in_maps, core_ids=list(range(B)))
    kernel.last_results = res
    return np.stack(
        [res.results[b]["out"].astype(np.float32) for b in range(B)], axis=0
    )


# revision 21
# speedup vs baseline: 1.2699x; 1.2699x over previous
"""Trainium2 Bass kernel for nn_Encoder_Spatio (gnn_message_passing).

Math (validated against the reference):
    h1 = ELU(x @ fc1_w.T + b1)
    h2 = ELU(h1 @ fc2_w.T + b2)
    probs[b,i,j] = sq[b,i] + sk[b,j] + c
where BN (eval) and the Q/K projections + mlp2 halves fold on the host into
    vq, vk in R^256,  c scalar:    sq = h2 @ vq,  sk = h2 @ vk  (+consts).

Sharding: pure data-parallel over B (8 batches -> 8 cores). Each core runs the
identical program on its own batch; no collectives.

Performance structure (error budget is rel 2e-2; measured ~4e-4):
  * fp32r matmuls: 1 PE cycle/row instead of fp32's 4. Inputs/weights land
    as f32 bytes through fp32r-typed DMAs (the DMA is the "rounded"
    producer the BIR verifier wants; the PE rounds internally).
  * fp16 output: the [2048,2048] score matrix goes to DRAM as fp16
    (8.4 MB instead of 16.8 MB -- the kernel is output-DMA-bound) and is
    upcast to fp32 on the host.
  * Layer biases ride into PSUM as rank-1 matmuls (stationary [1,128] bias
    row x moving ones row), so both oc-chunks of a layer live in one
    two-bank PSUM pair and every ELU op processes [128,1024] at once:
    exp and relu on Scalar (Pool/GpSimd tensor ops are software-emulated
    and ~15x slower AND stall concurrent DVE work, so GpSimd does nothing),
    fused (e-1) min r on Vector, which also rounds its fp32r output.
    ELU(z) = min(exp(z)-1, relu(z)) exactly.
  * t0 (the sk+c broadcast) is stored fp16 so the output emission
    tensor_scalar on DVE runs in its 2-byte 4x perf mode; sq stays a
    per-partition f32 scalar AP (exempt from the 2-byte rule).
  * Output ships triangularly: [512x512] quads as soon as a (row-block,
    column-slice) pair exists, then the last row-block group as four full
    [128x2048] rows. Weights land in ONE packed DMA issued FIRST on the
    sync ring (its hardware queue outranks the scalar ring's), then input
    slices 0-1; slices 2-3 ride the scalar ring.
"""

import sys

if "/opt/trn_rl_repo" not in sys.path:
    sys.path.insert(0, "/opt/trn_rl_repo")

import types

import numpy as np


def _ensure_axon_hooks():
    """concourse.bass_utils imports antenv.axon_hooks when tracing is
    requested; this image's antenv package lacks that submodule, which turns
    a skipped-trace fallback into a hard ImportError. Fill the hole with a
    None-hook stub (tracing degrades gracefully) if it's missing."""
    try:
        import antenv.axon_hooks  # noqa: F401
        return
    except ImportError:
        pass
    try:
        import antenv
    except ImportError:
        return
    mod = types.ModuleType("antenv.axon_hooks")
    mod._hook = None

    def set_axon_ntff_profile_hook(hook):
        mod._hook = hook

    def get_axon_ntff_profile_hook():
        return mod._hook

    mod.set_axon_ntff_profile_hook = set_axon_ntff_profile_hook
    mod.get_axon_ntff_profile_hook = get_axon_ntff_profile_hook
    sys.modules["antenv.axon_hooks"] = mod
    antenv.axon_hooks = mod


_ensure_axon_hooks()

from concourse import bass, tile, mybir
from concourse.bass_utils import run_bass_kernel_spmd

B, N, F = 8, 2048, 256      # batch, tokens, feature width (NIN == NHID == 256)
KC = F // 128               # feature chunks of 128 partitions
TS = 4                      # token slices
SW = N // TS                # slice width (512 = max fp32 matmul moving dim)
RBW = 128                   # output row-block width (partition dim)
NRB = N // RBW              # 16 row blocks
RB_PER_TS = SW // RBW       # row blocks per token slice

MM_DT = mybir.dt.float32r   # one-pass fp32 through the PE (4x fp32 rate)
OUT_DT = mybir.dt.float16   # device-side output precision (host upcasts)

F32 = mybir.dt.float32
AF = mybir.ActivationFunctionType
ALU = mybir.AluOpType

# packed-weights column layout (partition-major [128, WPK] f32)
WC_W1 = 0                   # KC chunks of F columns
WC_W2 = WC_W1 + KC * F
WC_VKB = WC_W2 + KC * F     # KC chunks of 128 columns
WC_VQ = WC_VKB + KC * 128   # 2*KC columns (vq chunk | zeros pairs)
WC_CB = WC_VQ + 2 * KC      # 1 column
WC_B1 = WC_CB + 1           # KC columns
WC_B2 = WC_B1 + KC
WPK = WC_B2 + KC


def _split_multiwaits(nc):
    """This walrus build lowers at most one sync-wait per instruction on some
    instruction classes (the TileContext exit drain trips it). Hoist extra
    waits onto preceding single-wait drains on the same engine."""
    for f in nc.m.functions:
        for bb in f.blocks:
            insts = list(bb.instructions)
            out = []
            changed = False
            for inst in insts:
                si = inst.sync_info
                if si is not None and si.on_wait and len(si.on_wait) > 1:
                    waits = list(si.on_wait)
                    for k, w in enumerate(waits[:-1]):
                        d = mybir.InstDrain(name=f"{inst.name}-ws{k}")
                        d.engine = inst.engine
                        d.sync_info = mybir.SyncInfo(on_wait=[w], on_update=[])
                        out.append(d)
                    inst.sync_info = mybir.SyncInfo(
                        on_wait=[waits[-1]], on_update=list(si.on_update)
                    )
                    changed = True
                out.append(inst)
            if changed:
                bb.instructions = out


def _build_program():
    nc = bass.Bass(trn_type="TRN2")

    xt_d = nc.dram_tensor("xt", [F, N], F32, kind="ExternalInput")
    wpk_d = nc.dram_tensor("wpk", [128, WPK], F32, kind="ExternalInput")
    out_d = nc.dram_tensor("out", [N, N], OUT_DT, kind="ExternalOutput")

    with tile.TileContext(nc) as tc:
        with (
            tc.tile_pool(name="wts", bufs=1) as wpool,
            tc.tile_pool(name="xin", bufs=1) as xpool,
            tc.tile_pool(name="eh", bufs=3) as epool,
            tc.tile_pool(name="rh", bufs=3) as rpool,
            tc.tile_pool(name="h1", bufs=3) as h1pool,
            tc.tile_pool(name="h2", bufs=3) as h2pool,
            tc.tile_pool(name="t0", bufs=1) as t0pool,
            tc.tile_pool(name="sqs", bufs=TS) as sqpool,
            tc.tile_pool(name="ob", bufs=8) as opool,
            tc.tile_pool(name="psmm", bufs=6, space="PSUM") as pspool,
            tc.tile_pool(name="pst0", bufs=1, space="PSUM") as t0ps_pool,
            tc.tile_pool(name="pssq", bufs=1, space="PSUM") as sqps_pool,
        ):
            # ---- packed weights FIRST on the sync ring (its queue has
            # priority and every matmul waits on this one transfer)
            wstg = wpool.tile([128, WPK], MM_DT, tag="wstg", name="wstg")
            nc.sync.dma_start(wstg[:], wpk_d[:].bitcast(MM_DT))
            w1 = [wstg[:, WC_W1 + k * F:WC_W1 + (k + 1) * F] for k in range(KC)]
            w2 = [wstg[:, WC_W2 + k * F:WC_W2 + (k + 1) * F] for k in range(KC)]
            vkb = [wstg[:, WC_VKB + k * 128:WC_VKB + (k + 1) * 128]
                   for k in range(KC)]
            vq = wstg[:, WC_VQ:WC_VQ + 2 * KC]
            cb = wstg[:, WC_CB:WC_CB + 1].bitcast(F32)
            b1 = wstg[:, WC_B1:WC_B1 + KC].bitcast(F32)
            b2 = wstg[:, WC_B2:WC_B2 + KC].bitcast(F32)

            # ---- input slices: 0-1 behind the weights on the sync ring,
            # 2-3 on the scalar ring; one DMA per slice, fp32r-typed
            xstg = {}
            for s, ring in ((0, nc.sync), (1, nc.sync),
                            (2, nc.scalar), (3, nc.scalar)):
                stg = xpool.tile([128, KC, SW], MM_DT, name=f"xs_{s}", tag=f"xs_{s}")
                ring.dma_start(
                    stg[:],
                    xt_d[:, s * SW:(s + 1) * SW].rearrange(
                        "(k p) c -> p k c", k=KC
                    ).bitcast(MM_DT),
                )
                xstg[s] = stg
            xs = {
                s: [xstg[s][:, k, :] for k in range(KC)] for s in range(TS)
            }

            # HAM warmup: dummy matmuls on memset tiles fill the PE during
            # the load window so the first real matmul runs fast
            dmw = wpool.tile([128, 128], F32, tag="dmw", name="dmw")
            dmx = wpool.tile([128, 256], F32, tag="dmx", name="dmx")
            nc.gpsimd.memset(dmw[:], 0.0)
            nc.gpsimd.memset(dmx[:], 0.0)
            dps = t0ps_pool.tile([128, SW], F32, name="dps", tag="pst0")
            for w_i in range(3):
                nc.tensor.matmul(dps[:, 0:256], dmw[:], dmx[:],
                                 start=(w_i == 0), stop=(w_i == 2))

            t0_full = t0pool.tile([128, N], OUT_DT, name="t0_full", tag="t0_full")
            t0 = []        # per-slice views into t0_full [128, SW]
            sq = []        # per-rowblock sq views [128, 1] (f32)
            h1s = {}       # slice -> [128, KC, SW] fp32r tile
            h2s = {}       # slice -> [128, KC, SW] fp32r tile

            def mlp_layer(w, bias, rhs, out_pool, tag):
                """One Linear+ELU layer for a token slice.
                ELU(z) = min(exp(z)-1, relu(z)) exactly: exp on Scalar, relu
                split Scalar/Vector (both read PSUM; the split keeps either
                queue from gating the score stages), fused (e-1) min r on
                Vector, which also rounds its fp32r output.
                rhs: list of KC [128, SW] fp32r APs. Returns the [128,KC,SW]
                ELU tile (chunk k = [:, k, :])."""
                h = out_pool.tile([128, KC, SW], MM_DT, tag=tag, name=tag)
                for oc in range(KC):
                    ps = pspool.tile([128, SW], F32, name="ps_mm", tag="psmm")
                    for k in range(KC):
                        nc.tensor.matmul(
                            ps[:],
                            w[k][:, oc * 128:(oc + 1) * 128],
                            rhs[k],
                            start=(k == 0),
                            stop=(k == KC - 1),
                        )
                    bia = bias[:, oc:oc + 1]
                    e = epool.tile([128, SW], F32, name="e_t")
                    r = rpool.tile([128, SW], F32, name="r_t")
                    nc.scalar.activation(e[:], ps[:], AF.Exp, bias=bia)
                    if oc == 0:
                        nc.scalar.activation(r[:], ps[:], AF.Relu, bias=bia)
                    else:
                        nc.vector.tensor_scalar(
                            r[:], ps[:], bia, 0.0, ALU.add, ALU.max
                        )
                    nc.vector.scalar_tensor_tensor(
                        h[:, oc, :], e[:], -1.0, r[:], ALU.add, ALU.min
                    )
                return h

            n_out = 0

            def emit_quad(q, s):
                """One output DMA covering row blocks 4q..4q+3 at column
                slice s: a [128, 4, 512] fp16 SBUF tile lands on the
                [512, 512] DRAM region in a single transfer. The adds run
                on DVE in its 2-byte 4x perf mode (t0 is fp16, sq a
                per-partition f32 scalar, which is exempt), with a few on
                Scalar to balance the engines."""
                nonlocal n_out
                ot = opool.tile([128, RB_PER_TS, SW], OUT_DT, name="out_t", tag="out_t")
                for rb in range(RB_PER_TS):
                    i = q * RB_PER_TS + rb
                    if n_out % 4 == 3:
                        nc.scalar.activation(
                            ot[:, rb, :], t0[s], AF.Identity, bias=sq[i]
                        )
                    else:
                        nc.vector.tensor_scalar(
                            ot[:, rb, :], t0[s], sq[i], None, ALU.add
                        )
                    n_out += 1
                dram = out_d[
                    q * RB_PER_TS * RBW:(q + 1) * RB_PER_TS * RBW,
                    s * SW:(s + 1) * SW,
                ].rearrange("(b p) c -> p b c", b=RB_PER_TS)
                # sync engine is nearly idle; keep output-DMA issue cost
                # off the busy compute engines
                nc.sync.dma_start(dram, ot[:])

            def stage_T0(s):
                """sk[j] + c for column slice s, broadcast to all partitions
                via the replicated-vk matmul, narrowed to fp16 by the Scalar
                copy that adds the constant."""
                pst = t0ps_pool.tile([128, SW], F32, name="ps_t0", tag="pst0")
                for k in range(KC):
                    nc.tensor.matmul(
                        pst[:], vkb[k][:], h2s[s][:, k, :],
                        start=(k == 0), stop=(k == KC - 1),
                    )
                t0s = t0_full[:, s * SW:(s + 1) * SW]
                nc.scalar.activation(t0s, pst[:], AF.Identity, bias=cb)
                t0.append(t0s)

            def stage_SQ(s):
                """sq for this slice's row blocks (fp32r ISA needs even
                moving/output dims, hence the zero-padded vq pairs). All
                row blocks of the slice land in one PSUM tile so a single
                DVE copy moves them to SBUF."""
                qps = sqps_pool.tile([128, 2 * RB_PER_TS], F32, name="qps")
                for rb in range(RB_PER_TS):
                    for k in range(KC):
                        nc.tensor.matmul(
                            qps[:, 2 * rb:2 * rb + 2],
                            h2s[s][:, k, rb * RBW:(rb + 1) * RBW],
                            vq[:, 2 * k:2 * k + 2],
                            start=(k == 0),
                            stop=(k == KC - 1),
                        )
                sqt = sqpool.tile(
                    [128, 2 * RB_PER_TS], F32, tag=f"sq_{s}", name=f"sq_{s}"
                )
                nc.vector.tensor_copy(sqt[:], qps[:])
                for rb in range(RB_PER_TS):
                    sq.append(sqt[:, 2 * rb:2 * rb + 1])

            def stage_A(s):
                h1s[s] = mlp_layer(w1, b1, xs.pop(s), h1pool, "h1")

            def stage_B(s):
                h2s[s] = mlp_layer(w2, b2, [h1s[s][:, k, :] for k in range(KC)],
                                   h2pool, "h2")
                h1s.pop(s)

            def stage_C(s):
                """Scores for slice s, then the triangular emission of every
                newly-possible (row-block quad, column slice) pair. The last
                slice instead ships its row blocks as full [128, 2048] rows
                (4 KB descriptors)."""
                stage_T0(s)
                stage_SQ(s)
                if s == TS - 1:
                    for q in range(TS - 1):
                        emit_quad(q, s)
                    for rb in range(RB_PER_TS):
                        i = s * RB_PER_TS + rb
                        orow = opool.tile(
                            [128, N], OUT_DT, name="out_row", tag="out_t"
                        )
                        if rb % 2 == 0:
                            nc.vector.tensor_scalar(
                                orow[:], t0_full[:], sq[i], None, ALU.add
                            )
                        else:
                            nc.scalar.activation(
                                orow[:], t0_full[:], AF.Identity, bias=sq[i]
                            )
                        nc.sync.dma_start(
                            out_d[i * RBW:(i + 1) * RBW, :], orow[:]
                        )
                else:
                    for q in range(s):
                        emit_quad(q, s)
                    for sp in range(s + 1):
                        emit_quad(s, sp)

            # PE work order: two A-stages lead before the first B so the PE
            # always has independent work queued while each stage's ELU
            # chain (Scalar exp/relu -> Vector min) drains; C stages follow
            # their B immediately so the output stream starts early.
            stage_A(0)
            stage_A(1)
            stage_A(2)
            stage_B(0)
            stage_A(3)
            stage_C(0)
            stage_B(1)
            stage_C(1)
            stage_B(2)
            stage_C(2)
            stage_B(3)
            stage_C(3)

    _split_multiwaits(nc)
    return nc


_prog_cache = {}


def _get_program():
    if "nc" not in _prog_cache:
        _prog_cache["nc"] = _build_program()
    return _prog_cache["nc"]


def kernel(**inputs):
    inp = np.asarray(inputs["inputs"], np.float32)        # [B, N, F]
    fc1_w = np.asarray(inputs["fc1_w"], np.float64)
    fc1_b = np.asarray(inputs["fc1_b"], np.float64)
    fc2_w = np.asarray(inputs["fc2_w"], np.float64)
    fc2_b = np.asarray(inputs["fc2_b"], np.float64)
    bn_g = np.asarray(inputs["bn_g"], np.float64)
    bn_b = np.asarray(inputs["bn_b"], np.float64)
    bn_mean = np.asarray(inputs["bn_mean"], np.float64)
    bn_var = np.asarray(inputs["bn_var"], np.float64)
    wq_w = np.asarray(inputs["wq_w"], np.float64)
    wq_b = np.asarray(inputs["wq_b"], np.float64)
    wk_w = np.asarray(inputs["wk_w"], np.float64)
    wk_b = np.asarray(inputs["wk_b"], np.float64)
    mlp2_w = np.asarray(inputs["mlp2_w"], np.float64)
    mlp2_b = np.asarray(inputs["mlp2_b"], np.float64)

    # Fold BN (eval) into the Q/K projections, then both projections and the
    # mlp2 halves into two R^F vectors + one scalar (exact linear algebra).
    D = wq_w.shape[0]
    s = bn_g / np.sqrt(bn_var + 1e-5)
    t = bn_b - bn_mean * s
    wqf = wq_w * s[None, :]
    bqf = wq_b + wq_w @ t
    wkf = wk_w * s[None, :]
    bkf = wk_b + wk_w @ t
    wk_half, wq_half = mlp2_w[0, :D], mlp2_w[0, D:]
    vq = wqf.T @ wq_half
    vk = wkf.T @ wk_half
    c_total = float(bqf @ wq_half + bkf @ wk_half + mlp2_b[0])

    # packed weights tile, mirroring the WC_* column layout
    wpk = np.zeros((128, WPK), np.float32)
    wpk[:, WC_W1:WC_W1 + KC * F] = np.concatenate(
        [fc1_w.T[k * 128:(k + 1) * 128, :] for k in range(KC)], axis=1
    )
    wpk[:, WC_W2:WC_W2 + KC * F] = np.concatenate(
        [fc2_w.T[k * 128:(k + 1) * 128, :] for k in range(KC)], axis=1
    )
    wpk[:, WC_VKB:WC_VKB + KC * 128] = np.concatenate(
        [np.tile(vk[k * 128:(k + 1) * 128, None], (1, 128)) for k in range(KC)],
        axis=1,
    )
    for k in range(KC):
        wpk[:, WC_VQ + 2 * k] = vq[k * 128:(k + 1) * 128]
    wpk[:, WC_CB] = c_total
    wpk[:, WC_B1:WC_B1 + KC] = fc1_b.reshape(KC, 128).T
    wpk[:, WC_B2:WC_B2 + KC] = fc2_b.reshape(KC, 128).T

    in_maps = [
        {"xt": np.ascontiguousarray(inp[b].T), "wpk": wpk} for b in range(B)
    ]

    nc = _get_program()
    res = run_bass_kernel_spmd(nc, in_maps, core_ids=list(range(B)))
    kernel.last_results = res
    return np.stack(
        [res.results[b]["out"].astype(np.float32) for b in range(B)], axis=0
    )
